# revision 12
# baseline (speedup 1.0000x reference)
"""Trainium2 Bass kernel for ExpanderLinearLayer (gather-mul-scatter_add).

Reformulation: out = input_ @ S + bias, where S[i, j] = sum of weight[k] over
all k with ind_in[k] == i and ind_out[k] == j.  S is built dense on the host
(52224 nnz into 1024x1024, ~0.5% of the device FLOPs) and the device runs a
dense bf16 matmul, data-parallel over the batch across 8 NeuronCores.

v3 (vs v2 at ~32.1us):
  * chunk 0 split into c0a=[bias|x_0|s_0[0:3]] (912 cols) + c0b=s_0[3:8]:
    the first real matmul is gated by c0a's 233KB instead of the full
    397KB chunk -> k=0 pass starts ~1.5us earlier.  NWARM trimmed to
    match (warmups only need to cover until c0a lands; the PE ramp
    penalty of starting at mid p-state is smaller than idling).
  * tail: per-m merged (6,m),(7,m) finalize (as v2) but outputs ship in
    4 paired DMAs ([0:2],[2:4],[4:6],[6:8]) issued right after the
    corresponding drains, so the 1MB output stream starts ~3us earlier
    and the last 256KB block lands ~1.5us after the final matmul
    instead of ~3.3us.

Per core (batch shard of 512 rows), the 1024-long contraction dim is split
into 8 chunks of 128.  Chunk k of the merged input tensor `xs` holds
[x_k | s_k] side by side so ONE DMA (one semaphore lane) delivers everything
the chunk-k matmuls need.  Chunk 0 additionally carries the 8 fp32 bias
columns (16 bf16 columns, bitcast on device).

  chunk k (k>0) at cols [16 + k*1536, 16 + (k+1)*1536):   [x_k | s_k]
  chunk 0 at cols [0, 16 + 1536):                         [bias | x_0 | s_0]
      x_k[p, n] = input_[c*512+n, k*128+p]   (n < 512)
      s_k[p, m] = S[k*128+p, m]              (m < 1024)
      bias[p, m] = bias[m*128+p]             (m < 8, fp32)
  o  [128, 8*512] bf16:  o[p, m*512+n] = out[c*512+n, m*128+p]
"""

import os
import numpy as np

try:
    from concourse import bacc, bass, mybir
    from concourse.tile import TileContext
    from concourse.bass_utils import run_bass_kernel_spmd
except ImportError:  # fresh dir without PYTHONPATH
    import sys

    sys.path.insert(0, "/opt/trn_rl_repo")
    from concourse import bacc, bass, mybir
    from concourse.tile import TileContext
    from concourse.bass_utils import run_bass_kernel_spmd

P = 128
B = 4096
D = 1024
NCORES = 8
BS = B // NCORES      # 512 batch rows per core
KO = D // P           # 8 contraction chunks
MO = D // P           # 8 output tiles
CW = BS + D           # 1536 columns per merged chunk
MOH = 2 * MO          # bf16 cols holding the fp32 bias at chunk-0 head
NWARM = 7             # PE warm-up matmuls during the DMA head
S0A = 3               # s_0 m-tiles carried in the first (head) DMA
C0A = MOH + BS + S0A * P   # 912 cols in the head DMA
S1A = 3               # s_1 m-tiles in chunk 1's first DMA
C1A = BS + S1A * P    # 896 cols in chunk 1's first DMA

F32 = mybir.dt.float32
BF16 = mybir.dt.bfloat16
BF16_NP = mybir.dt.np(BF16)

_NC_CACHE = {}
LAST_RESULTS = None


def _build_nc():
    # Bacc (not raw Bass): its compile() pass legalizes multi-wait
    # instructions (event semaphores, matmul waits moved to ldweights) —
    # TPB instructions encode only a single sync-wait.
    nc = bacc.Bacc("TRN2", target_bir_lowering=False)
    xs_d = nc.declare_dram_parameter("xs", [P, MOH + KO * CW], BF16, isOutput=False)
    o_d = nc.declare_dram_parameter("o", [P, MO * BS], BF16, isOutput=True)

    with TileContext(nc) as tc:
        with (
            tc.tile_pool(name="cs", bufs=1) as cpool,
            tc.tile_pool(name="ob", bufs=1) as opool,
            tc.tile_pool(name="ps", bufs=1, space="PSUM") as pspool,
        ):
            # Single HWDGE ring (sync): FIFO ordering means early chunks
            # drain at full HBM rate before later chunks start.  The ring
            # has a ~1.5us first-packet cold latency after the doorbell;
            # the head DMA (c0a) is kept small so the first k=0 matmuls
            # are gated by 233KB, not the whole 397KB chunk 0.
            c0a = cpool.tile([P, C0A], BF16, tag="c0a", name="c0a")
            nc.sync.dma_start(c0a, xs_d[:, :C0A])
            c0b = cpool.tile([P, CW + MOH - C0A], BF16, tag="c0b", name="c0b")
            nc.sync.dma_start(c0b, xs_d[:, C0A:CW + MOH])
            # chunk 1 split the same way: it is consumed while the ring
            # is still ramping (~250GB/s for its first ~2.5us), so gate
            # the k=1 matmuls on as few bytes as possible.
            off1 = MOH + CW
            c1a = cpool.tile([P, C1A], BF16, tag="c1a", name="c1a")
            nc.sync.dma_start(c1a, xs_d[:, off1:off1 + C1A])
            c1b = cpool.tile([P, CW - C1A], BF16, tag="c1b", name="c1b")
            nc.sync.dma_start(c1b, xs_d[:, off1 + C1A:off1 + CW])
            chunks = []
            for k in range(2, KO):
                off = MOH + k * CW
                ct = cpool.tile([P, CW], BF16, tag=f"c{k}", name=f"c{k}")
                nc.sync.dma_start(ct, xs_d[:, off:off + CW])
                chunks.append(ct)

            # fp32 bias columns live at the head of chunk 0
            bias_ap = c0a[:, :MOH].bitcast(F32)

            def chunk_x(k):
                if k == 0:
                    return c0a[:, MOH:MOH + BS]
                if k == 1:
                    return c1a[:, :BS]
                return chunks[k - 2][:, :BS]

            def chunk_s(k, m):
                if k == 0:
                    if m < S0A:
                        base = MOH + BS
                        return c0a[:, base + m * P:base + (m + 1) * P]
                    return c0b[:, (m - S0A) * P:(m - S0A + 1) * P]
                if k == 1:
                    if m < S1A:
                        return c1a[:, BS + m * P:BS + (m + 1) * P]
                    return c1b[:, (m - S1A) * P:(m - S1A + 1) * P]
                base = BS
                return chunks[k - 2][:, base + m * P:base + (m + 1) * P]

            psums = [
                pspool.tile([P, BS], F32, tag=f"ps{m}", name=f"ps{m}")
                for m in range(MO)
            ]
            out_sb = opool.tile([P, MO, BS], BF16, tag="out")

            # PE warm-up: HAM clock-gates a cold PE to 1.2GHz; ~3us of
            # CONTINUOUS activity unlocks 2.4GHz.  Warmups bridge the gap
            # from loop entry (~7.5us) until c0a's data lands (~10.4us,
            # the ring itself ramps slowly for its first ~2us) so the PE
            # never idles before the real stream — an idle gap would
            # re-throttle HAM and run the first real matmuls at half
            # speed, costing more than the gap itself.
            wu = cpool.tile([P, BS], BF16, tag="wu")
            nc.gpsimd.memset(wu[:, :], 0.0)
            for _ in range(NWARM):
                nc.tensor.matmul(
                    psums[0], lhsT=wu[:, :P], rhs=wu[:, :],
                    start=True, stop=True,
                )

            # k-passes 0..5: psum[m] += s_k[m].T @ x_k
            for k in range(KO - 2):
                rhs = chunk_x(k)
                for m in range(MO):
                    nc.tensor.matmul(
                        psums[m],
                        lhsT=chunk_s(k, m),
                        rhs=rhs,
                        start=(k == 0),
                        stop=False,
                    )
            # merged tail passes 6+7: finalize psum[m] every ~0.43us and
            # drain it immediately; vector/scalar alternate so the two
            # PSUM readers run in parallel on different banks.  Outputs
            # ship in pairs right after the odd drain lands, spreading
            # the 1MB output stream across the tail instead of piling it
            # after the last matmul.
            for m in range(MO):
                nc.tensor.matmul(
                    psums[m], lhsT=chunk_s(KO - 2, m), rhs=chunk_x(KO - 2),
                    start=False, stop=False,
                )
                nc.tensor.matmul(
                    psums[m], lhsT=chunk_s(KO - 1, m), rhs=chunk_x(KO - 1),
                    start=False, stop=True,
                )
                if m % 2 == 0:
                    nc.vector.tensor_scalar_add(
                        out_sb[:, m], psums[m], bias_ap[:, m:m + 1]
                    )
                else:
                    nc.scalar.add(out_sb[:, m], psums[m], bias_ap[:, m:m + 1])

            out_r = o_d[:, :].rearrange("p (m n) -> p m n", m=MO)
            for m0 in range(0, MO - 2, 2):
                nc.sync.dma_start(out_r[:, m0:m0 + 2], out_sb[:, m0:m0 + 2])
            # last two blocks ship separately so the final (critical-path)
            # transfer is only 128KB
            nc.sync.dma_start(out_r[:, 6:7], out_sb[:, 6:7])
            nc.sync.dma_start(out_r[:, 7:8], out_sb[:, 7:8])

    nc.finalize()
    return nc


def _get_nc():
    if "nc" not in _NC_CACHE:
        _NC_CACHE["nc"] = _build_nc()
    return _NC_CACHE["nc"]


def kernel(input_, weight, bias, ind_in, ind_out):
    global LAST_RESULTS
    input_ = np.asarray(input_, dtype=np.float32)
    weight = np.asarray(weight, dtype=np.float32)
    bias = np.asarray(bias, dtype=np.float32)
    ind_in = np.asarray(ind_in, dtype=np.int64)
    ind_out = np.asarray(ind_out, dtype=np.int64)

    # Dense scatter matrix S.
    S = np.zeros((D, D), np.float32)
    np.add.at(S, (ind_in, ind_out), weight)
    S16 = S.astype(BF16_NP)
    # fp32 bias [128, 8] viewed as bf16 [128, 16] for the merged DMA
    b_l = np.ascontiguousarray(bias.reshape(MO, P).T).view(BF16_NP)

    in_maps = []
    for c in range(NCORES):
        xT = np.ascontiguousarray(
            input_[c * BS:(c + 1) * BS].T
        ).astype(BF16_NP)  # [1024, 512]
        xs_l = np.empty((P, MOH + KO * CW), BF16_NP)
        xs_l[:, :MOH] = b_l
        for k in range(KO):
            rows = slice(k * P, (k + 1) * P)
            off = MOH + k * CW
            xs_l[:, off:off + BS] = xT[rows]
            xs_l[:, off + BS:off + CW] = S16[rows]
        in_maps.append({"xs": xs_l})

    nc = _get_nc()
    res = run_bass_kernel_spmd(
        nc,
        in_maps,
        core_ids=list(range(NCORES)),
        trace=bool(int(os.environ.get("KERNEL_TRACE", "0"))),
    )
    LAST_RESULTS = res

    outs = []
    for c in range(NCORES):
        o = np.asarray(res.results[c]["o"], dtype=np.float32)
        outT = o.reshape(P, MO, BS).transpose(1, 0, 2).reshape(D, BS)
        outs.append(outT.T)
    return np.ascontiguousarray(np.concatenate(outs, axis=0))


# revision 14
# speedup vs baseline: 1.0110x; 1.0110x over previous
"""Trainium2 Bass kernel for ExpanderLinearLayer (gather-mul-scatter_add).

Reformulation: out = input_ @ S + bias, where S[i, j] = sum of weight[k] over
all k with ind_in[k] == i and ind_out[k] == j.  S is built dense on the host
(52224 nnz into 1024x1024, ~0.5% of the device FLOPs) and the device runs a
dense bf16 matmul, data-parallel over the batch across 8 NeuronCores.

v3 (vs v2 at ~32.1us):
  * chunk 0 split into c0a=[bias|x_0|s_0[0:3]] (912 cols) + c0b=s_0[3:8]:
    the first real matmul is gated by c0a's 233KB instead of the full
    397KB chunk -> k=0 pass starts ~1.5us earlier.  NWARM trimmed to
    match (warmups only need to cover until c0a lands; the PE ramp
    penalty of starting at mid p-state is smaller than idling).
  * tail: per-m merged (6,m),(7,m) finalize (as v2) but outputs ship in
    4 paired DMAs ([0:2],[2:4],[4:6],[6:8]) issued right after the
    corresponding drains, so the 1MB output stream starts ~3us earlier
    and the last 256KB block lands ~1.5us after the final matmul
    instead of ~3.3us.

Per core (batch shard of 512 rows), the 1024-long contraction dim is split
into 8 chunks of 128.  Chunk k of the merged input tensor `xs` holds
[x_k | s_k] side by side so ONE DMA (one semaphore lane) delivers everything
the chunk-k matmuls need.  Chunk 0 additionally carries the 8 fp32 bias
columns (16 bf16 columns, bitcast on device).

  chunk k (k>0) at cols [16 + k*1536, 16 + (k+1)*1536):   [x_k | s_k]
  chunk 0 at cols [0, 16 + 1536):                         [bias | x_0 | s_0]
      x_k[p, n] = input_[c*512+n, k*128+p]   (n < 512)
      s_k[p, m] = S[k*128+p, m]              (m < 1024)
      bias[p, m] = bias[m*128+p]             (m < 8, fp32)
  o  [128, 8*512] bf16:  o[p, m*512+n] = out[c*512+n, m*128+p]
"""

import os
import numpy as np

try:
    from concourse import bacc, bass, mybir
    from concourse.tile import TileContext
    from concourse.bass_utils import run_bass_kernel_spmd
except ImportError:  # fresh dir without PYTHONPATH
    import sys

    sys.path.insert(0, "/opt/trn_rl_repo")
    from concourse import bacc, bass, mybir
    from concourse.tile import TileContext
    from concourse.bass_utils import run_bass_kernel_spmd

P = 128
B = 4096
D = 1024
NCORES = 8
BS = B // NCORES      # 512 batch rows per core
KO = D // P           # 8 contraction chunks
MO = D // P           # 8 output tiles
CW = BS + D           # 1536 columns per merged chunk
MOH = 2 * MO          # bf16 cols holding the fp32 bias at chunk-0 head
NWARM = 7             # PE warm-up matmuls during the DMA head
S0A = 3               # s_0 m-tiles carried in the first (head) DMA
C0A = MOH + BS + S0A * P   # 912 cols in the head DMA
S1A = 3               # s_1 m-tiles in chunk 1's first DMA
C1A = BS + S1A * P    # 896 cols in chunk 1's first DMA

F32 = mybir.dt.float32
BF16 = mybir.dt.bfloat16
BF16_NP = mybir.dt.np(BF16)

_NC_CACHE = {}
LAST_RESULTS = None


def _build_nc():
    # Bacc (not raw Bass): its compile() pass legalizes multi-wait
    # instructions (event semaphores, matmul waits moved to ldweights) —
    # TPB instructions encode only a single sync-wait.
    nc = bacc.Bacc("TRN2", target_bir_lowering=False)
    xs_d = nc.declare_dram_parameter("xs", [P, MOH + KO * CW], BF16, isOutput=False)
    o_d = nc.declare_dram_parameter("o", [P, MO * BS], BF16, isOutput=True)

    with TileContext(nc) as tc:
        with (
            tc.tile_pool(name="cs", bufs=1) as cpool,
            tc.tile_pool(name="ob", bufs=1) as opool,
            tc.tile_pool(name="ps", bufs=1, space="PSUM") as pspool,
        ):
            # Single HWDGE ring (sync): FIFO ordering means early chunks
            # drain at full HBM rate before later chunks start.  The ring
            # has a ~1.5us first-packet cold latency after the doorbell;
            # the head DMA (c0a) is kept small so the first k=0 matmuls
            # are gated by 233KB, not the whole 397KB chunk 0.
            c0a = cpool.tile([P, C0A], BF16, tag="c0a", name="c0a")
            nc.sync.dma_start(c0a, xs_d[:, :C0A])
            c0b = cpool.tile([P, CW + MOH - C0A], BF16, tag="c0b", name="c0b")
            nc.sync.dma_start(c0b, xs_d[:, C0A:CW + MOH])
            # chunk 1 split the same way: it is consumed while the ring
            # is still ramping (~250GB/s for its first ~2.5us), so gate
            # the k=1 matmuls on as few bytes as possible.
            off1 = MOH + CW
            c1a = cpool.tile([P, C1A], BF16, tag="c1a", name="c1a")
            nc.sync.dma_start(c1a, xs_d[:, off1:off1 + C1A])
            c1b = cpool.tile([P, CW - C1A], BF16, tag="c1b", name="c1b")
            nc.sync.dma_start(c1b, xs_d[:, off1 + C1A:off1 + CW])
            chunks = []
            for k in range(2, KO):
                off = MOH + k * CW
                ct = cpool.tile([P, CW], BF16, tag=f"c{k}", name=f"c{k}")
                nc.sync.dma_start(ct, xs_d[:, off:off + CW])
                chunks.append(ct)
            # Warm the scalar HWDGE ring (Q10) with a throwaway 512B
            # read now, so the critical last output block pays no
            # ~1.3us cold-start when it ships on Q10 at the very end.
            q10wu = cpool.tile([P, 2], BF16, tag="q10wu")
            nc.scalar.dma_start(q10wu, xs_d[:, :2])

            # fp32 bias columns live at the head of chunk 0
            bias_ap = c0a[:, :MOH].bitcast(F32)

            def chunk_x(k):
                if k == 0:
                    return c0a[:, MOH:MOH + BS]
                if k == 1:
                    return c1a[:, :BS]
                return chunks[k - 2][:, :BS]

            def chunk_s(k, m):
                if k == 0:
                    if m < S0A:
                        base = MOH + BS
                        return c0a[:, base + m * P:base + (m + 1) * P]
                    return c0b[:, (m - S0A) * P:(m - S0A + 1) * P]
                if k == 1:
                    if m < S1A:
                        return c1a[:, BS + m * P:BS + (m + 1) * P]
                    return c1b[:, (m - S1A) * P:(m - S1A + 1) * P]
                base = BS
                return chunks[k - 2][:, base + m * P:base + (m + 1) * P]

            psums = [
                pspool.tile([P, BS], F32, tag=f"ps{m}", name=f"ps{m}")
                for m in range(MO)
            ]
            out_sb = opool.tile([P, MO, BS], BF16, tag="out")

            # PE warm-up: HAM clock-gates a cold PE to 1.2GHz; ~3us of
            # CONTINUOUS activity unlocks 2.4GHz.  Warmups bridge the gap
            # from loop entry (~7.5us) until c0a's data lands (~10.4us,
            # the ring itself ramps slowly for its first ~2us) so the PE
            # never idles before the real stream — an idle gap would
            # re-throttle HAM and run the first real matmuls at half
            # speed, costing more than the gap itself.
            wu = cpool.tile([P, BS], BF16, tag="wu")
            nc.gpsimd.memset(wu[:, :], 0.0)
            for _ in range(NWARM):
                nc.tensor.matmul(
                    psums[0], lhsT=wu[:, :P], rhs=wu[:, :],
                    start=True, stop=True,
                )

            # k-passes 0..5: psum[m] += s_k[m].T @ x_k
            for k in range(KO - 2):
                rhs = chunk_x(k)
                for m in range(MO):
                    nc.tensor.matmul(
                        psums[m],
                        lhsT=chunk_s(k, m),
                        rhs=rhs,
                        start=(k == 0),
                        stop=False,
                    )
            # merged tail passes 6+7: finalize psum[m] every ~0.43us and
            # drain it immediately; vector/scalar alternate so the two
            # PSUM readers run in parallel on different banks.  Outputs
            # ship in pairs right after the odd drain lands, spreading
            # the 1MB output stream across the tail instead of piling it
            # after the last matmul.  For m6/m7 the k=7 matmul and the
            # drain are split into 256-col halves so both drain engines
            # work each tile in parallel and the last half-drain starts
            # the moment the last matmul retires.
            out_r = o_d[:, :].rearrange("p (m n) -> p m n", m=MO)
            H = BS // 2
            for m in range(MO - 2):
                nc.tensor.matmul(
                    psums[m], lhsT=chunk_s(KO - 2, m), rhs=chunk_x(KO - 2),
                    start=False, stop=False,
                )
                nc.tensor.matmul(
                    psums[m], lhsT=chunk_s(KO - 1, m), rhs=chunk_x(KO - 1),
                    start=False, stop=True,
                )
                if m % 2 == 0:
                    nc.vector.tensor_scalar_add(
                        out_sb[:, m], psums[m], bias_ap[:, m:m + 1]
                    )
                else:
                    nc.scalar.add(out_sb[:, m], psums[m], bias_ap[:, m:m + 1])
                if m % 2 == 1:
                    nc.sync.dma_start(
                        out_r[:, m - 1:m + 1], out_sb[:, m - 1:m + 1]
                    )
            for m in (MO - 2, MO - 1):
                nc.tensor.matmul(
                    psums[m], lhsT=chunk_s(KO - 2, m), rhs=chunk_x(KO - 2),
                    start=False, stop=False,
                )
                x7 = chunk_x(KO - 1)
                nc.tensor.matmul(
                    psums[m][:, :H], lhsT=chunk_s(KO - 1, m), rhs=x7[:, :H],
                    start=False, stop=True,
                )
                nc.tensor.matmul(
                    psums[m][:, H:], lhsT=chunk_s(KO - 1, m), rhs=x7[:, H:],
                    start=False, stop=True,
                )
                nc.scalar.add(
                    out_sb[:, m, :H], psums[m][:, :H], bias_ap[:, m:m + 1]
                )
                nc.vector.tensor_scalar_add(
                    out_sb[:, m, H:], psums[m][:, H:], bias_ap[:, m:m + 1]
                )
            nc.sync.dma_start(out_r[:, 6:7], out_sb[:, 6:7])
            # the last 128KB rides the warmed scalar ring, in parallel
            # with [6:7] (and anything still draining) on the sync ring
            nc.scalar.dma_start(out_r[:, 7:8], out_sb[:, 7:8])

    nc.finalize()
    return nc


def _get_nc():
    if "nc" not in _NC_CACHE:
        _NC_CACHE["nc"] = _build_nc()
    return _NC_CACHE["nc"]


def kernel(input_, weight, bias, ind_in, ind_out):
    global LAST_RESULTS
    input_ = np.asarray(input_, dtype=np.float32)
    weight = np.asarray(weight, dtype=np.float32)
    bias = np.asarray(bias, dtype=np.float32)
    ind_in = np.asarray(ind_in, dtype=np.int64)
    ind_out = np.asarray(ind_out, dtype=np.int64)

    # Dense scatter matrix S.
    S = np.zeros((D, D), np.float32)
    np.add.at(S, (ind_in, ind_out), weight)
    S16 = S.astype(BF16_NP)
    # fp32 bias [128, 8] viewed as bf16 [128, 16] for the merged DMA
    b_l = np.ascontiguousarray(bias.reshape(MO, P).T).view(BF16_NP)

    in_maps = []
    for c in range(NCORES):
        xT = np.ascontiguousarray(
            input_[c * BS:(c + 1) * BS].T
        ).astype(BF16_NP)  # [1024, 512]
        xs_l = np.empty((P, MOH + KO * CW), BF16_NP)
        xs_l[:, :MOH] = b_l
        for k in range(KO):
            rows = slice(k * P, (k + 1) * P)
            off = MOH + k * CW
            xs_l[:, off:off + BS] = xT[rows]
            xs_l[:, off + BS:off + CW] = S16[rows]
        in_maps.append({"xs": xs_l})

    nc = _get_nc()
    res = run_bass_kernel_spmd(
        nc,
        in_maps,
        core_ids=list(range(NCORES)),
        trace=bool(int(os.environ.get("KERNEL_TRACE", "0"))),
    )
    LAST_RESULTS = res

    outs = []
    for c in range(NCORES):
        o = np.asarray(res.results[c]["o"], dtype=np.float32)
        outT = o.reshape(P, MO, BS).transpose(1, 0, 2).reshape(D, BS)
        outs.append(outT.T)
    return np.ascontiguousarray(np.concatenate(outs, axis=0))


# revision 18
# speedup vs baseline: 1.1700x; 1.1573x over previous
"""Trainium2 Bass kernel for ExpanderLinearLayer (gather-mul-scatter_add).

Reformulation: out = input_ @ S + bias, where S[i, j] = sum of weight[k] over
all k with ind_in[k] == i and ind_out[k] == j.  S is built dense on the host
(52224 nnz into 1024x1024, ~0.5% of the device FLOPs) and the device runs a
dense bf16 matmul, data-parallel over the batch across 8 NeuronCores.

v9 (vs v2 at ~32.1us; ~30.7-31.2us depending on device thermal state):
  * chunk 0 split into c0a=[bias|x_0|s_0[0:3]] (233KB) + c0b=s_0[3:8],
    and chunk 1 split the same way: the single sync HWDGE ring ramps
    slowly (~150-250GB/s for its first ~2.5us, ~360GB/s after), so the
    first k-pass matmuls gate on as few bytes as possible.  First real
    matmul ~10.3us (vs 11.4us with a monolithic chunk 0).
  * NWARM=7 full-width warmups sized to end EXACTLY when c0a lands: any
    PE idle gap re-throttles HAM and the first real matmuls then run at
    427-634ns instead of 216ns, costing more than the gap.  (Retune
    NWARM if the head timing changes.)
  * tail: per-m merged (6,m),(7,m) finalize every ~0.43us, drain
    immediately (vector/scalar alternate -> parallel PSUM banks),
    outputs ship in paired DMAs ([0:2],[2:4],[4:6],[6:7]) as drains
    land, spreading the ~300GB/s output stream across the tail.  The
    final 128KB is issued by the scalar engine right after its own m7
    drain onto the PRE-WARMED scalar ring (Q10), in parallel with the
    sync ring — no cross-engine hop, no doorbell queueing.
  * measured dead ends: fp8 (3.7% err > 2e-2 gate), dual-ring INPUT
    (rings share HBM BW; +1us), half-drains of one psum tile on V+S
    (Tile framework serializes them), DMA-from-PSUM (unsupported).
  * remaining budget (good state): ~0.8us entry + ~3.6us DMA head
    (ring cold+ramp) + 13.8us matmul floor + ~4.3us drain/ship/sem
    tail + ~7.5us fixed framework exit (host loop-exit handshake).

Per core (batch shard of 512 rows), the 1024-long contraction dim is split
into 8 chunks of 128.  Chunk k of the merged input tensor `xs` holds
[x_k | s_k] side by side so ONE DMA (one semaphore lane) delivers everything
the chunk-k matmuls need.  Chunk 0 additionally carries the 8 fp32 bias
columns (16 bf16 columns, bitcast on device).

  chunk k (k>0) at cols [16 + k*1536, 16 + (k+1)*1536):   [x_k | s_k]
  chunk 0 at cols [0, 16 + 1536):                         [bias | x_0 | s_0]
      x_k[p, n] = input_[c*512+n, k*128+p]   (n < 512)
      s_k[p, m] = S[k*128+p, m]              (m < 1024)
      bias[p, m] = bias[m*128+p]             (m < 8, fp32)
  o  [128, 8*512] bf16:  o[p, m*512+n] = out[c*512+n, m*128+p]
"""

import os
import numpy as np

try:
    from concourse import bacc, bass, mybir
    from concourse.tile import TileContext
    from concourse.bass_utils import run_bass_kernel_spmd
except ImportError:  # fresh dir without PYTHONPATH
    import sys

    sys.path.insert(0, "/opt/trn_rl_repo")
    from concourse import bacc, bass, mybir
    from concourse.tile import TileContext
    from concourse.bass_utils import run_bass_kernel_spmd

P = 128
B = 4096
D = 1024
NCORES = 8
BS = B // NCORES      # 512 batch rows per core
KO = D // P           # 8 contraction chunks
MO = D // P           # 8 output tiles
CW = BS + D           # 1536 columns per merged chunk
MOH = 2 * MO          # bf16 cols holding the fp32 bias at chunk-0 head
NWARM = 7             # PE warm-up matmuls during the DMA head
S0A = 3               # s_0 m-tiles carried in the first (head) DMA
C0A = MOH + BS + S0A * P   # 912 cols in the head DMA
S1A = 3               # s_1 m-tiles in chunk 1's first DMA
C1A = BS + S1A * P    # 896 cols in chunk 1's first DMA

F32 = mybir.dt.float32
BF16 = mybir.dt.bfloat16
BF16_NP = mybir.dt.np(BF16)

_NC_CACHE = {}
LAST_RESULTS = None


def _build_nc():
    # Bacc (not raw Bass): its compile() pass legalizes multi-wait
    # instructions (event semaphores, matmul waits moved to ldweights) —
    # TPB instructions encode only a single sync-wait.
    nc = bacc.Bacc("TRN2", target_bir_lowering=False)
    xs_d = nc.declare_dram_parameter("xs", [P, MOH + KO * CW], BF16, isOutput=False)
    o_d = nc.declare_dram_parameter("o", [P, MO * BS], BF16, isOutput=True)

    with TileContext(nc) as tc:
        with (
            tc.tile_pool(name="cs", bufs=1) as cpool,
            tc.tile_pool(name="ob", bufs=1) as opool,
            tc.tile_pool(name="ps", bufs=1, space="PSUM") as pspool,
        ):
            # Single HWDGE ring (sync): FIFO ordering means early chunks
            # drain at full HBM rate before later chunks start.  The ring
            # has a ~1.5us first-packet cold latency after the doorbell;
            # the head DMA (c0a) is kept small so the first k=0 matmuls
            # are gated by 233KB, not the whole 397KB chunk 0.
            c0a = cpool.tile([P, C0A], BF16, tag="c0a", name="c0a")
            nc.sync.dma_start(c0a, xs_d[:, :C0A])
            c0b = cpool.tile([P, CW + MOH - C0A], BF16, tag="c0b", name="c0b")
            nc.sync.dma_start(c0b, xs_d[:, C0A:CW + MOH])
            # chunk 1 split the same way: it is consumed while the ring
            # is still ramping (~250GB/s for its first ~2.5us), so gate
            # the k=1 matmuls on as few bytes as possible.
            off1 = MOH + CW
            c1a = cpool.tile([P, C1A], BF16, tag="c1a", name="c1a")
            nc.sync.dma_start(c1a, xs_d[:, off1:off1 + C1A])
            c1b = cpool.tile([P, CW - C1A], BF16, tag="c1b", name="c1b")
            nc.sync.dma_start(c1b, xs_d[:, off1 + C1A:off1 + CW])
            chunks = []
            for k in range(2, KO):
                off = MOH + k * CW
                ct = cpool.tile([P, CW], BF16, tag=f"c{k}", name=f"c{k}")
                nc.sync.dma_start(ct, xs_d[:, off:off + CW])
                chunks.append(ct)
            # Warm the scalar HWDGE ring (Q10) with a throwaway 512B
            # read now, so the critical last output block pays no
            # ~1.3us cold-start when it ships on Q10 at the very end.
            q10wu = cpool.tile([P, 2], BF16, tag="q10wu")
            nc.scalar.dma_start(q10wu, xs_d[:, :2])

            # fp32 bias columns live at the head of chunk 0
            bias_ap = c0a[:, :MOH].bitcast(F32)

            def chunk_x(k):
                if k == 0:
                    return c0a[:, MOH:MOH + BS]
                if k == 1:
                    return c1a[:, :BS]
                return chunks[k - 2][:, :BS]

            def chunk_s(k, m):
                if k == 0:
                    if m < S0A:
                        base = MOH + BS
                        return c0a[:, base + m * P:base + (m + 1) * P]
                    return c0b[:, (m - S0A) * P:(m - S0A + 1) * P]
                if k == 1:
                    if m < S1A:
                        return c1a[:, BS + m * P:BS + (m + 1) * P]
                    return c1b[:, (m - S1A) * P:(m - S1A + 1) * P]
                base = BS
                return chunks[k - 2][:, base + m * P:base + (m + 1) * P]

            psums = [
                pspool.tile([P, BS], F32, tag=f"ps{m}", name=f"ps{m}")
                for m in range(MO)
            ]
            out_sb = opool.tile([P, MO, BS], BF16, tag="out")

            # PE warm-up: HAM clock-gates a cold PE to 1.2GHz; ~3us of
            # CONTINUOUS activity unlocks 2.4GHz.  Warmups bridge the gap
            # from loop entry (~7.5us) until c0a's data lands (~10.4us,
            # the ring itself ramps slowly for its first ~2us) so the PE
            # never idles before the real stream — an idle gap would
            # re-throttle HAM and run the first real matmuls at half
            # speed, costing more than the gap itself.
            wu = cpool.tile([P, BS], BF16, tag="wu")
            nc.gpsimd.memset(wu[:, :], 0.0)
            for _ in range(NWARM):
                nc.tensor.matmul(
                    psums[0], lhsT=wu[:, :P], rhs=wu[:, :],
                    start=True, stop=True,
                )

            # k-passes 0..5: psum[m] += s_k[m].T @ x_k
            for k in range(KO - 2):
                rhs = chunk_x(k)
                for m in range(MO):
                    nc.tensor.matmul(
                        psums[m],
                        lhsT=chunk_s(k, m),
                        rhs=rhs,
                        start=(k == 0),
                        stop=False,
                    )
            # merged tail passes 6+7: finalize psum[m] every ~0.43us and
            # drain it immediately; vector/scalar alternate so the two
            # PSUM readers run in parallel on different banks.  Outputs
            # ship in pairs right after the odd drain lands, spreading
            # the 1MB output stream across the tail instead of piling it
            # after the last matmul.
            out_r = o_d[:, :].rearrange("p (m n) -> p m n", m=MO)
            for m in range(MO):
                nc.tensor.matmul(
                    psums[m], lhsT=chunk_s(KO - 2, m), rhs=chunk_x(KO - 2),
                    start=False, stop=False,
                )
                nc.tensor.matmul(
                    psums[m], lhsT=chunk_s(KO - 1, m), rhs=chunk_x(KO - 1),
                    start=False, stop=True,
                )
                if m % 2 == 0:
                    nc.vector.tensor_scalar_add(
                        out_sb[:, m], psums[m], bias_ap[:, m:m + 1]
                    )
                else:
                    nc.scalar.add(out_sb[:, m], psums[m], bias_ap[:, m:m + 1])
                if m % 2 == 1 and m < MO - 1:
                    nc.sync.dma_start(
                        out_r[:, m - 1:m + 1], out_sb[:, m - 1:m + 1]
                    )
            nc.sync.dma_start(out_r[:, 6:7], out_sb[:, 6:7])
            # the last 128KB is issued by the SCALAR engine itself right
            # after its m7 drain (no cross-engine hop, no queueing behind
            # [6:7] on the sync ring) and rides the pre-warmed Q10 ring.
            nc.scalar.dma_start(out_r[:, 7:8], out_sb[:, 7:8])

    nc.finalize()
    return nc


def _get_nc():
    if "nc" not in _NC_CACHE:
        _NC_CACHE["nc"] = _build_nc()
    return _NC_CACHE["nc"]


def kernel(input_, weight, bias, ind_in, ind_out):
    global LAST_RESULTS
    input_ = np.asarray(input_, dtype=np.float32)
    weight = np.asarray(weight, dtype=np.float32)
    bias = np.asarray(bias, dtype=np.float32)
    ind_in = np.asarray(ind_in, dtype=np.int64)
    ind_out = np.asarray(ind_out, dtype=np.int64)

    # Dense scatter matrix S.
    S = np.zeros((D, D), np.float32)
    np.add.at(S, (ind_in, ind_out), weight)
    S16 = S.astype(BF16_NP)
    # fp32 bias [128, 8] viewed as bf16 [128, 16] for the merged DMA
    b_l = np.ascontiguousarray(bias.reshape(MO, P).T).view(BF16_NP)

    in_maps = []
    for c in range(NCORES):
        xT = np.ascontiguousarray(
            input_[c * BS:(c + 1) * BS].T
        ).astype(BF16_NP)  # [1024, 512]
        xs_l = np.empty((P, MOH + KO * CW), BF16_NP)
        xs_l[:, :MOH] = b_l
        for k in range(KO):
            rows = slice(k * P, (k + 1) * P)
            off = MOH + k * CW
            xs_l[:, off:off + BS] = xT[rows]
            xs_l[:, off + BS:off + CW] = S16[rows]
        in_maps.append({"xs": xs_l})

    nc = _get_nc()
    res = run_bass_kernel_spmd(
        nc,
        in_maps,
        core_ids=list(range(NCORES)),
        trace=bool(int(os.environ.get("KERNEL_TRACE", "0"))),
    )
    LAST_RESULTS = res

    outs = []
    for c in range(NCORES):
        o = np.asarray(res.results[c]["o"], dtype=np.float32)
        outT = o.reshape(P, MO, BS).transpose(1, 0, 2).reshape(D, BS)
        outs.append(outT.T)
    return np.ascontiguousarray(np.concatenate(outs, axis=0))
